# revision 35
# baseline (speedup 1.0000x reference)
"""Multi-head attention (B=8, S=2048, D=512, H=8) on 8 Trainium2 NeuronCores.

Strategy: pure data parallelism — one batch element per core, no collectives.

Per-core device pipeline (all matmuls fp16 with fp32 PSUM accumulation):
  1. Projections: qT/kT in transposed layout [e, s] (attention contracts
     dk on partitions), v in natural [s, e] layout augmented with a ones
     column per head (the PV matmul then also produces softmax denominators).
     Inputs arrive pre-transposed from host as X^T [c, s] fp16.  Projection
     issue is interleaved with attention so the ScalarE exp stream starts
     as soon as qT/kT for the first head pair exist (~20us in) instead of
     after the whole projection phase.
  2. Attention per (head-pair, s-block of 512): the two heads of an e-tile
     live on partitions 0-63 / 64-127, so their K=64 score matmuls are
     issued back-to-back as PE row-tiles (tile_position (0,0)/(64,0)) and
     run concurrently into one [128, 2, 512] PSUM tile (2 banks,
     double-buffered).  One FD=1024 ScalarE exp covers the pair
     (PSUM->SBUF fp16), one DVE multiply applies the 0/1 mask to both
     heads via a zero-stride broadcast AP, then two PV matmuls with [V|1]
     stationary accumulate outT rows + denominators in PSUM.  The
     pipeline is ScalarE-bound (33.5M exps/core at 1 elem/lane/cycle);
     PE and DVE have slack.
  3. Denominator rows staged to SBUF, DVE reciprocal, partition-broadcast
     via a DRAM bounce, in-place normalize, final projection with Wo.T
     (issued after the next s-block's first pair so the exp stream never
     waits on it), bias, DMA out.

Softmax note: reference softmax(where(mask==0, -1e30, s)) == exp(s)*mask
normalized — scores are O(1) so no max-subtraction is needed, and the 0/1
mask is exact in fp16. Scale 1/sqrt(dk)=1/8 is folded into Wq/bq on host.
"""
import numpy as np

import concourse.bacc as bacc
import concourse.bass as bass
import concourse.mybir as mybir
import concourse.tile as tile
from concourse.bass_utils import run_bass_kernel_spmd

B, S, D, H, DK = 8, 2048, 512, 8, 64
P = 128            # partition tile
NET = D // P       # 4 e-tiles (contraction chunks / head pairs)
NST = S // P       # 16 s-tiles / j-tiles
SCW = 512          # matmul moving free dim
NSC = S // SCW     # 4
SHW = 512          # attention s-block width
NSH = S // SHW     # 4

f32 = mybir.dt.float32
fp16 = mybir.dt.float16

_CACHE: dict = {}


def _bcast(ap, n):
    """Repeat a [P, w] AP n times along a new middle dim (stride 0)."""
    return bass.AP(tensor=ap.tensor, offset=ap.offset,
                   ap=[ap.ap[0], [0, n]] + ap.ap[1:])


def _build():
    nc = bacc.Bacc("TRN2", target_bir_lowering=False, debug=False)

    d_xq = nc.dram_tensor("xq", [D, S], fp16, kind="ExternalInput")
    d_xk = nc.dram_tensor("xk", [D, S], fp16, kind="ExternalInput")
    d_xv = nc.dram_tensor("xv", [D, S], fp16, kind="ExternalInput")
    # mask chunked by s-block on host: [NSH, S(j), SHW]
    d_mskT = nc.dram_tensor("mskT", [NSH, S, SHW], fp16, kind="ExternalInput")
    d_wq = nc.dram_tensor("wq", [D, D], fp16, kind="ExternalInput")  # Wq.T/8
    d_wk = nc.dram_tensor("wk", [D, D], fp16, kind="ExternalInput")  # Wk.T
    d_wv = nc.dram_tensor("wv", [D, D], fp16, kind="ExternalInput")  # Wv.T
    d_wo = nc.dram_tensor("wo", [D, D], fp16, kind="ExternalInput")  # Wo.T
    d_bq = nc.dram_tensor("bq", [D], f32, kind="ExternalInput")      # bq/8
    d_bk = nc.dram_tensor("bk", [D], f32, kind="ExternalInput")
    d_bv = nc.dram_tensor("bv", [D], f32, kind="ExternalInput")
    d_bo = nc.dram_tensor("bo", [D], f32, kind="ExternalInput")
    d_out = nc.dram_tensor("out", [S, D], f32, kind="ExternalOutput")
    d_rec = nc.dram_tensor("rec_dram", [H, S], f32)

    Exp = mybir.ActivationFunctionType.Exp

    with tile.TileContext(nc) as tc, \
         tc.tile_pool(name="persist", bufs=1) as persist, \
         tc.tile_pool(name="maskp", bufs=2) as maskp, \
         tc.tile_pool(name="projx", bufs=3) as projx, \
         tc.tile_pool(name="projw", bufs=3) as projw, \
         tc.tile_pool(name="attn", bufs=4) as attn, \
         tc.tile_pool(name="ps", bufs=1, space="PSUM") as psp:

        qT = persist.tile([P, NET, S], fp16)             # [e%128, et, s]
        kT = persist.tile([P, NET, S], fp16)
        v_aug = persist.tile([P, NST, H, DK + 1], fp16)  # [j%128, jt, h, d|1]
        outT = persist.tile([P, NET, S], fp16)           # [hd%128, et, s]
        denom = persist.tile([P, NSH, 64], f32)
        bq_sb = persist.tile([P, NET], f32)
        bk_sb = persist.tile([P, NET], f32)
        bv_bc = persist.tile([P, D], f32)
        wo_sb = persist.tile([P, NET, D], fp16)
        bo_bc = persist.tile([P, D], f32)
        warm = persist.tile([P, 2], f32)

        # small constants / biases first, then the exp table preload runs
        # while the big input DMAs stream.
        nc.sync.dma_start(out=bq_sb, in_=d_bq.ap().rearrange("(cc p) -> p cc", p=P))
        nc.sync.dma_start(out=bk_sb, in_=d_bk.ap().rearrange("(cc p) -> p cc", p=P))
        nc.sync.dma_start(
            out=bv_bc,
            in_=bass.AP(tensor=d_bv.ap().tensor, offset=0, ap=[[0, P], [1, D]]))
        nc.vector.memset(v_aug[:, :, :, DK:DK + 1], 1.0)
        nc.scalar.activation(warm[:, 0:1], bq_sb[:, 0:1], Exp)  # table preload

        # ---------------- input DMAs (issue order = criticality) --------
        # q and k first (gate the first scores), then v streamed JIT in
        # (cc, st-quarter) chunks so early PV matmuls never wait on the
        # whole xv transfer, then the first mask block.
        x_sbs, w_sbs, x_aps = [], [], []
        for which, (d_x, d_w) in enumerate(
                [(d_xq, d_wq), (d_xk, d_wk), (d_xv, d_wv)]):
            w_sb = projw.tile([P, NET, D], fp16, tag="w", name=f"w{which}")
            x_sb = projx.tile([P, NET, S], fp16, tag="x", name=f"x{which}")
            x_sbs.append(x_sb)
            w_sbs.append(w_sb)
            x_aps.append(d_x.ap().rearrange("(cc p) s -> p cc s", p=P))

        def dma_w(which):
            nc.sync.dma_start(
                out=w_sbs[which],
                in_=[d_wq, d_wk, d_wv][which].ap().rearrange(
                    "(cc p) e -> p cc e", p=P))

        def dma_x(which, lo, hi):
            for cc in range(NET):
                nc.sync.dma_start(out=x_sbs[which][:, cc, lo:hi],
                                  in_=x_aps[which][:, cc, lo:hi])

        m0 = maskp.tile([P, NST, SHW], fp16, tag="m", name="m0")
        # only q(et0, sc0) gates the first scores: 1MB of xq first, then
        # all of xk; xv quarters interleaved with mask rows so PV matmuls
        # and mask multiplies both stream JIT; deferred xq after.
        dma_w(0)
        dma_x(0, 0, SCW)
        dma_w(1)
        dma_x(1, 0, S)
        dma_w(2)
        msk0_ap = d_mskT.ap()[0].rearrange("(jt p) w -> p jt w", p=P)
        for q4 in range(4):
            dma_x(2, q4 * SCW, (q4 + 1) * SCW)
            nc.sync.dma_start(out=m0[:, q4 * 4:(q4 + 1) * 4, :],
                              in_=msk0_ap[:, q4 * 4:(q4 + 1) * 4, :])
        dma_x(0, SCW, S)
        nc.sync.dma_start(
            out=wo_sb, in_=d_wo.ap().rearrange("(cc p) e -> p cc e", p=P))
        nc.sync.dma_start(
            out=bo_bc,
            in_=bass.AP(tensor=d_bo.ap().tensor, offset=0, ap=[[0, P], [1, D]]))

        def load_mask(sh):
            m = maskp.tile([P, NST, SHW], fp16, tag="m", name=f"m{sh}")
            nc.sync.dma_start(
                out=m, in_=d_mskT.ap()[sh].rearrange("(jt p) w -> p jt w", p=P))
            return m

        def proj_qk(which, et, scs=None):
            dst = qT if which == 0 else kT
            bias = bq_sb if which == 0 else bk_sb
            for sc in (range(NSC) if scs is None else scs):
                ps_t = psp.tile([P, SCW], f32, tag="pp", bufs=2, name="ps_t")
                for cc in range(NET):
                    nc.tensor.matmul(
                        ps_t,
                        w_sbs[which][:, cc, et * P:(et + 1) * P],
                        x_sbs[which][:, cc, sc * SCW:(sc + 1) * SCW],
                        start=(cc == 0), stop=(cc == NET - 1))
                nc.vector.tensor_scalar_add(
                    dst[:, et, sc * SCW:(sc + 1) * SCW], ps_t,
                    bias[:, et:et + 1])

        def proj_v(st_lo, st_hi):
            for st in range(st_lo, st_hi):
                ps_t = psp.tile([P, SCW], f32, tag="pp", bufs=2, name="ps_t")
                for cc in range(NET):
                    nc.tensor.matmul(
                        ps_t,
                        x_sbs[2][:, cc, st * P:(st + 1) * P],
                        w_sbs[2][:, cc, :],
                        start=(cc == 0), stop=(cc == NET - 1))
                nc.vector.tensor_add(
                    v_aug[:, st, :, 0:DK],
                    ps_t.rearrange("p (h d) -> p h d", h=H),
                    bv_bc.rearrange("p (h d) -> p h d", h=H))

        def end_pair_thunks(sh, et, pvs):
            c0 = sh * SHW

            def cast(hh):
                ro = hh * DK
                nc.vector.tensor_copy(
                    outT[ro:ro + DK, et, c0:c0 + SHW], pvs[0:DK, hh, :])

            def dstage():
                dst_t = attn.tile([65, 2, SCW], f32, tag="dst", bufs=1,
                                  name="dst_t")
                nc.vector.tensor_copy(dst_t[64:65, :, :], pvs[64:65, :, :])
                nc.gpsimd.dma_start(
                    out=denom[et * 32:et * 32 + 16, sh, :],
                    in_=dst_t[64:65, :, :])

            def recb():
                rec = attn.tile([16, 64], f32, tag="rec", bufs=2, name="rec")
                nc.vector.reciprocal(rec, denom[et * 32:et * 32 + 16, sh, :])
                nc.sync.dma_start(
                    out=d_rec.ap()[2 * et:2 * et + 2, c0:c0 + SHW], in_=rec)

            def norm():
                rb = attn.tile([P, SHW], f32, tag="rb", bufs=2, name="rb")
                nc.gpsimd.dma_start(
                    out=rb[0:64, :],
                    in_=bass.AP(tensor=d_rec.ap().tensor,
                                offset=(2 * et) * S + c0,
                                ap=[[0, 64], [1, SHW]]))
                nc.gpsimd.dma_start(
                    out=rb[64:128, :],
                    in_=bass.AP(tensor=d_rec.ap().tensor,
                                offset=(2 * et + 1) * S + c0,
                                ap=[[0, 64], [1, SHW]]))
                nc.vector.tensor_mul(outT[:, et, c0:c0 + SHW],
                                     outT[:, et, c0:c0 + SHW], rb)

            return [lambda: cast(0), lambda: cast(1), dstage, recb, norm]

        def final_proj_st(st):
            ps_f = psp.tile([P, D], f32, tag="pp", bufs=2, name="ps_f")
            for cc in range(NET):
                nc.tensor.matmul(
                    ps_f,
                    outT[:, cc, st * P:(st + 1) * P],
                    wo_sb[:, cc, :],
                    start=(cc == 0), stop=(cc == NET - 1))
            o_sb = attn.tile([P, D], f32, tag="os", bufs=2, name="o_sb")
            nc.vector.tensor_add(o_sb, ps_f, bo_bc)
            nc.sync.dma_start(
                out=d_out.ap()[st * P:(st + 1) * P, :], in_=o_sb)

        def emit_rest(sc_ps, sh, et, jt, pvs, i):
            if jt == 0:
                pvs = psp.tile([65, 2, SCW], f32, tag="pv", bufs=1,
                               name="pv2")
            ex = attn.tile([P, 2, SHW], fp16, tag="ex", bufs=6, name="ex")
            nc.scalar.activation(ex, sc_ps, Exp)  # FD=1024
            pb = attn.tile([P, 2, SHW], fp16, tag="pb", bufs=8, name="pb")
            nc.vector.tensor_mul(pb, ex, _bcast(masks[sh][:, jt, :], 2))
            for hh in range(2):
                nc.tensor.matmul(
                    pvs[:, hh, :], v_aug[:, jt, 2 * et + hh, :],
                    pb[:, hh, :],
                    start=(jt == 0), stop=(jt == NST - 1))
            if jt == NST - 1:
                # spread the end-of-pair DVE burst over the next items so
                # the ex/pb rings (and thus the exp stream) never back up
                for off, th in zip([1, 2, 3, 5, 7],
                                   end_pair_thunks(sh, et, pvs)):
                    add_hook(i + off, th)
            return pvs

        # ---------------- issue schedule ----------------
        # Software-pipelined flat loop over all (sh, pair, jt): scores(i)
        # are issued BEFORE exp/mask/PV of item i-1 so the next pair's
        # scores never serialize behind the previous pair's drain.  Late
        # projections and final projections are spread one PSUM-group at
        # a time into the PE slack under the ScalarE-bound stream.
        proj_qk(0, 0, [0])      # only q(et0, sc0) gates the first scores
        proj_qk(1, 0)           # k et0, all s-chunks
        masks = {0: m0}

        # deferred work queue: thunks emitting ~1us of PE work each,
        # keyed by the flat item index before which they must be issued.
        items = [(sh, et, jt)
                 for sh in range(NSH) for et in range(NET)
                 for jt in range(NST)]
        hooks = {}

        def add_hook(idx, fn, *args):
            hooks.setdefault(idx, []).append((fn, args))

        def _pg(which, et, sc):
            dst = qT if which == 0 else kT
            bias = bq_sb if which == 0 else bk_sb
            ps_t = psp.tile([P, SCW], f32, tag="pp", bufs=2, name="ps_t")
            for cc in range(NET):
                nc.tensor.matmul(
                    ps_t,
                    w_sbs[which][:, cc, et * P:(et + 1) * P],
                    x_sbs[which][:, cc, sc * SCW:(sc + 1) * SCW],
                    start=(cc == 0), stop=(cc == NET - 1))
            nc.vector.tensor_scalar_add(
                dst[:, et, sc * SCW:(sc + 1) * SCW], ps_t,
                bias[:, et:et + 1])

        def load_mask_into(sh):
            masks[sh] = load_mask(sh)

        # v projection streamed one st-group per item (JIT ahead of the
        # PV matmuls, which lag scores by one item)
        for st in range(NST):
            add_hook(st, lambda s=st: proj_v(s, s + 1))
        # k (all chunks) + q (first s-chunk) for pair et must complete
        # before item (0,et,0); spread over the preceding pair's items.
        # q's s-chunks for later sh blocks are deferred until just before
        # that block starts.
        for et in range(1, NET):
            base = et * NST - 14
            add_hook(base, lambda e=et: _pg(0, e, 0))
            for g in range(4):
                add_hook(base + 2 * g + 1, lambda e=et, s=g: _pg(1, e, s))
        for sh in range(1, NSH):
            for et in range(NET):
                add_hook(sh * NET * NST - 14 + 3 * et,
                         lambda e=et, s=sh: _pg(0, e, s))
        # mask prefetch mid-way through the previous block
        for sh in range(1, NSH):
            add_hook((sh - 1) * NET * NST + 2 * NST, load_mask_into, sh)
        # final projection of block sh-1 spread over block sh's first pair
        # (starting several items in, so the rb bounce has completed)
        for sh in range(1, NSH):
            base = sh * NET * NST + 10
            for g, st in enumerate(range((sh - 1) * 4, sh * 4)):
                add_hook(base + 2 * g, final_proj_st, st)

        prev = None
        pvs = None
        for i, (sh, et, jt) in enumerate(items):
            for fn, args in hooks.get(i, []):
                fn(*args)
            c0 = sh * SHW
            sc_ps = psp.tile([P, 2, SHW], f32, tag="sc", bufs=2,
                             name="sc_ps")
            for hh in range(2):   # K=64 row-tiled pair, runs concurrent
                nc.tensor.matmul(
                    sc_ps[:, hh, :],
                    kT[hh * DK:(hh + 1) * DK, et, jt * P:(jt + 1) * P],
                    qT[hh * DK:(hh + 1) * DK, et, c0:c0 + SHW],
                    start=True, stop=True)
            if prev is not None:
                pvs = emit_rest(*prev, pvs, i)
            prev = (sc_ps, sh, et, jt)
        emit_rest(*prev, pvs, len(items))
        for i in range(len(items), len(items) + 12):  # drain tail hooks
            for fn, args in hooks.get(i, []):
                fn(*args)
        for st in range(NST - 4, NST):
            final_proj_st(st)

    nc.compile()
    return nc

    nc.compile()
    return nc


def _get_nc():
    if "nc" not in _CACHE:
        _CACHE["nc"] = _build()
    return _CACHE["nc"]


def _preprocess(Q, K, V, mask, Wq, bq, Wk, bk, Wv, bv, Wo, bo):
    """Host-side sharding + layout marshalling (per-core input dicts)."""
    mT = np.ascontiguousarray(np.asarray(mask)[0, 0].T).astype(np.float16)
    # chunk columns by s-block: [NSH, S(j), SHW]
    mTc = np.ascontiguousarray(mT.reshape(S, NSH, SHW).transpose(1, 0, 2))
    wq_h = np.ascontiguousarray(np.asarray(Wq).T / 8.0).astype(np.float16)
    wk_h = np.ascontiguousarray(np.asarray(Wk).T).astype(np.float16)
    wv_h = np.ascontiguousarray(np.asarray(Wv).T).astype(np.float16)
    wo_h = np.ascontiguousarray(np.asarray(Wo).T).astype(np.float16)
    bq_h = np.asarray(bq, dtype=np.float32) / 8.0
    bk_h = np.asarray(bk, dtype=np.float32)
    bv_h = np.asarray(bv, dtype=np.float32)
    bo_h = np.asarray(bo, dtype=np.float32)
    Q, K, V = np.asarray(Q), np.asarray(K), np.asarray(V)
    in_maps = []
    for b in range(B):
        in_maps.append({
            "xq": np.ascontiguousarray(Q[b].T).astype(np.float16),
            "xk": np.ascontiguousarray(K[b].T).astype(np.float16),
            "xv": np.ascontiguousarray(V[b].T).astype(np.float16),
            "mskT": mTc,
            "wq": wq_h, "wk": wk_h, "wv": wv_h, "wo": wo_h,
            "bq": bq_h, "bk": bk_h, "bv": bv_h, "bo": bo_h,
        })
    return in_maps


def run(inputs: dict, trace: bool = False):
    nc = _get_nc()
    in_maps = _preprocess(**inputs)
    res = run_bass_kernel_spmd(nc, in_maps, core_ids=list(range(B)), trace=trace)
    outp = np.stack([res.results[b]["out"] for b in range(B)], axis=0)
    return outp.astype(np.float32), res


def kernel(**inputs) -> np.ndarray:
    outp, _ = run(inputs, trace=False)
    return outp


# revision 39
# speedup vs baseline: 1.1630x; 1.1630x over previous
"""Multi-head attention (B=8, S=2048, D=512, H=8) on 8 Trainium2 NeuronCores.

Strategy: pure data parallelism — one batch element per core, no collectives.

Per-core device pipeline (all matmuls fp16 with fp32 PSUM accumulation):
  1. Projections: qT/kT in transposed layout [e, s] (attention contracts
     dk on partitions), v in natural [s, e] layout augmented with a ones
     column per head (the PV matmul then also produces softmax denominators).
     Inputs arrive pre-transposed from host as X^T [c, s] fp16.  Projection
     issue is interleaved with attention so the ScalarE exp stream starts
     as soon as qT/kT for the first head pair exist (~20us in) instead of
     after the whole projection phase.
  2. Attention per (head-pair, s-block of 512): the two heads of an e-tile
     live on partitions 0-63 / 64-127, so their K=64 score matmuls are
     issued back-to-back as PE row-tiles (tile_position (0,0)/(64,0)) and
     run concurrently into one [128, 2, 512] PSUM tile (2 banks,
     double-buffered).  One FD=1024 ScalarE exp covers the pair
     (PSUM->SBUF fp16), one DVE multiply applies the 0/1 mask to both
     heads via a zero-stride broadcast AP, then two PV matmuls with [V|1]
     stationary accumulate outT rows + denominators in PSUM.  The
     pipeline is ScalarE-bound (33.5M exps/core at 1 elem/lane/cycle);
     PE and DVE have slack.
  3. Denominator rows staged to SBUF, DVE reciprocal, partition-broadcast
     via a DRAM bounce, in-place normalize, final projection with Wo.T
     (issued after the next s-block's first pair so the exp stream never
     waits on it), bias, DMA out.

Softmax note: reference softmax(where(mask==0, -1e30, s)) == exp(s)*mask
normalized — scores are O(1) so no max-subtraction is needed, and the 0/1
mask is exact in fp16. Scale 1/sqrt(dk)=1/8 is folded into Wq/bq on host.
"""
import numpy as np

import concourse.bacc as bacc
import concourse.bass as bass
import concourse.mybir as mybir
import concourse.tile as tile
from concourse.bass_utils import run_bass_kernel_spmd

B, S, D, H, DK = 8, 2048, 512, 8, 64
P = 128            # partition tile
NET = D // P       # 4 e-tiles (contraction chunks / head pairs)
NST = S // P       # 16 s-tiles / j-tiles
SCW = 512          # matmul moving free dim
NSC = S // SCW     # 4
SHW = 512          # attention s-block width
NSH = S // SHW     # 4

f32 = mybir.dt.float32
fp16 = mybir.dt.float16

_CACHE: dict = {}


def _bcast(ap, n):
    """Repeat a [P, w] AP n times along a new middle dim (stride 0)."""
    return bass.AP(tensor=ap.tensor, offset=ap.offset,
                   ap=[ap.ap[0], [0, n]] + ap.ap[1:])


def _build():
    nc = bacc.Bacc("TRN2", target_bir_lowering=False, debug=False)

    d_xq = nc.dram_tensor("xq", [D, S], fp16, kind="ExternalInput")
    d_xk = nc.dram_tensor("xk", [D, S], fp16, kind="ExternalInput")
    d_xv = nc.dram_tensor("xv", [D, S], fp16, kind="ExternalInput")
    # mask chunked by s-block on host: [NSH, S(j), SHW]
    d_mskT = nc.dram_tensor("mskT", [NSH, S, SHW], fp16, kind="ExternalInput")
    d_wq = nc.dram_tensor("wq", [D, D], fp16, kind="ExternalInput")  # Wq.T/8
    d_wk = nc.dram_tensor("wk", [D, D], fp16, kind="ExternalInput")  # Wk.T
    d_wv = nc.dram_tensor("wv", [D, D], fp16, kind="ExternalInput")  # Wv.T
    d_wo = nc.dram_tensor("wo", [D, D], fp16, kind="ExternalInput")  # Wo.T
    d_bq = nc.dram_tensor("bq", [D], f32, kind="ExternalInput")      # bq/8
    d_bk = nc.dram_tensor("bk", [D], f32, kind="ExternalInput")
    d_bv = nc.dram_tensor("bv", [D], f32, kind="ExternalInput")
    d_bo = nc.dram_tensor("bo", [D], f32, kind="ExternalInput")
    d_out = nc.dram_tensor("out", [S, D], f32, kind="ExternalOutput")
    d_rec = nc.dram_tensor("rec_dram", [H, S], f32)

    Exp = mybir.ActivationFunctionType.Exp

    with tile.TileContext(nc) as tc, \
         tc.tile_pool(name="persist", bufs=1) as persist, \
         tc.tile_pool(name="maskp", bufs=2) as maskp, \
         tc.tile_pool(name="projx", bufs=3) as projx, \
         tc.tile_pool(name="projw", bufs=3) as projw, \
         tc.tile_pool(name="attn", bufs=4) as attn, \
         tc.tile_pool(name="ps", bufs=1, space="PSUM") as psp:

        qT = persist.tile([P, NET, S], fp16)             # [e%128, et, s]
        kT = persist.tile([P, NET, S], fp16)
        v_aug = persist.tile([P, NST, H, DK + 1], fp16)  # [j%128, jt, h, d|1]
        outT = persist.tile([P, NET, S], fp16)           # [hd%128, et, s]
        denom = persist.tile([P, NSH, 64], f32)
        bq_sb = persist.tile([P, NET], f32)
        bk_sb = persist.tile([P, NET], f32)
        bv_bc = persist.tile([P, D], f32)
        wo_sb = persist.tile([P, NET, D], fp16)
        bo_bc = persist.tile([P, D], f32)
        warm = persist.tile([P, 2], f32)

        # small constants / biases first, then the exp table preload runs
        # while the big input DMAs stream.
        nc.sync.dma_start(out=bq_sb, in_=d_bq.ap().rearrange("(cc p) -> p cc", p=P))
        nc.sync.dma_start(out=bk_sb, in_=d_bk.ap().rearrange("(cc p) -> p cc", p=P))
        nc.sync.dma_start(
            out=bv_bc,
            in_=bass.AP(tensor=d_bv.ap().tensor, offset=0, ap=[[0, P], [1, D]]))
        nc.vector.memset(v_aug[:, :, :, DK:DK + 1], 1.0)
        nc.scalar.activation(warm[:, 0:1], bq_sb[:, 0:1], Exp)  # table preload

        # ---------------- input DMAs (issue order = criticality) --------
        # q and k first (gate the first scores), then v streamed JIT in
        # (cc, st-quarter) chunks so early PV matmuls never wait on the
        # whole xv transfer, then the first mask block.
        x_sbs, w_sbs, x_aps = [], [], []
        for which, (d_x, d_w) in enumerate(
                [(d_xq, d_wq), (d_xk, d_wk), (d_xv, d_wv)]):
            w_sb = projw.tile([P, NET, D], fp16, tag="w", name=f"w{which}")
            x_sb = projx.tile([P, NET, S], fp16, tag="x", name=f"x{which}")
            x_sbs.append(x_sb)
            w_sbs.append(w_sb)
            x_aps.append(d_x.ap().rearrange("(cc p) s -> p cc s", p=P))

        def dma_w(which):
            nc.sync.dma_start(
                out=w_sbs[which],
                in_=[d_wq, d_wk, d_wv][which].ap().rearrange(
                    "(cc p) e -> p cc e", p=P))

        def dma_x(which, lo, hi):
            for cc in range(NET):
                nc.sync.dma_start(out=x_sbs[which][:, cc, lo:hi],
                                  in_=x_aps[which][:, cc, lo:hi])

        m0 = maskp.tile([P, NST, SHW], fp16, tag="m", name="m0")
        # only q(et0, sc0) gates the first scores: 1MB of xq first, then
        # all of xk; xv quarters interleaved with mask rows so PV matmuls
        # and mask multiplies both stream JIT; deferred xq after.
        dma_w(0)
        dma_x(0, 0, SCW)
        dma_w(1)
        dma_x(1, 0, S)
        dma_w(2)
        msk0_ap = d_mskT.ap()[0].rearrange("(jt p) w -> p jt w", p=P)
        for q4 in range(4):
            dma_x(2, q4 * SCW, (q4 + 1) * SCW)
            nc.sync.dma_start(out=m0[:, q4 * 4:(q4 + 1) * 4, :],
                              in_=msk0_ap[:, q4 * 4:(q4 + 1) * 4, :])
        dma_x(0, SCW, S)
        nc.sync.dma_start(
            out=wo_sb, in_=d_wo.ap().rearrange("(cc p) e -> p cc e", p=P))
        nc.sync.dma_start(
            out=bo_bc,
            in_=bass.AP(tensor=d_bo.ap().tensor, offset=0, ap=[[0, P], [1, D]]))

        def load_mask(sh):
            m = maskp.tile([P, NST, SHW], fp16, tag="m", name=f"m{sh}")
            nc.sync.dma_start(
                out=m, in_=d_mskT.ap()[sh].rearrange("(jt p) w -> p jt w", p=P))
            return m

        def proj_qk(which, et, scs=None):
            dst = qT if which == 0 else kT
            bias = bq_sb if which == 0 else bk_sb
            for sc in (range(NSC) if scs is None else scs):
                ps_t = psp.tile([P, SCW], f32, tag="pp", bufs=2, name="ps_t")
                for cc in range(NET):
                    nc.tensor.matmul(
                        ps_t,
                        w_sbs[which][:, cc, et * P:(et + 1) * P],
                        x_sbs[which][:, cc, sc * SCW:(sc + 1) * SCW],
                        start=(cc == 0), stop=(cc == NET - 1))
                nc.vector.tensor_scalar_add(
                    dst[:, et, sc * SCW:(sc + 1) * SCW], ps_t,
                    bias[:, et:et + 1])

        def proj_v(st_lo, st_hi):
            for st in range(st_lo, st_hi):
                ps_t = psp.tile([P, SCW], f32, tag="pp", bufs=2, name="ps_t")
                for cc in range(NET):
                    nc.tensor.matmul(
                        ps_t,
                        x_sbs[2][:, cc, st * P:(st + 1) * P],
                        w_sbs[2][:, cc, :],
                        start=(cc == 0), stop=(cc == NET - 1))
                nc.vector.tensor_add(
                    v_aug[:, st, :, 0:DK],
                    ps_t.rearrange("p (h d) -> p h d", h=H),
                    bv_bc.rearrange("p (h d) -> p h d", h=H))

        def end_pair_thunks(sh, et, pvs):
            c0 = sh * SHW

            def cast(hh):
                ro = hh * DK
                nc.vector.tensor_copy(
                    outT[ro:ro + DK, et, c0:c0 + SHW], pvs[0:DK, hh, :])

            def dstage():
                dst_t = attn.tile([65, 2, SCW], f32, tag="dst", bufs=1,
                                  name="dst_t")
                nc.vector.tensor_copy(dst_t[64:65, :, :], pvs[64:65, :, :])
                nc.gpsimd.dma_start(
                    out=denom[et * 32:et * 32 + 16, sh, :],
                    in_=dst_t[64:65, :, :])

            def recb():
                rec = attn.tile([16, 64], f32, tag="rec", bufs=2, name="rec")
                nc.vector.reciprocal(rec, denom[et * 32:et * 32 + 16, sh, :])
                nc.sync.dma_start(
                    out=d_rec.ap()[2 * et:2 * et + 2, c0:c0 + SHW], in_=rec)

            def norm():
                rb = attn.tile([P, SHW], f32, tag="rb", bufs=2, name="rb")
                nc.gpsimd.dma_start(
                    out=rb[0:64, :],
                    in_=bass.AP(tensor=d_rec.ap().tensor,
                                offset=(2 * et) * S + c0,
                                ap=[[0, 64], [1, SHW]]))
                nc.gpsimd.dma_start(
                    out=rb[64:128, :],
                    in_=bass.AP(tensor=d_rec.ap().tensor,
                                offset=(2 * et + 1) * S + c0,
                                ap=[[0, 64], [1, SHW]]))
                nc.vector.tensor_mul(outT[:, et, c0:c0 + SHW],
                                     outT[:, et, c0:c0 + SHW], rb)

            return [lambda: cast(0), lambda: cast(1), dstage, recb, norm]

        def final_proj_st(st, tag="pp"):
            ps_f = psp.tile([P, D], f32, tag=tag, bufs=2, name="ps_f")
            for cc in range(NET):
                nc.tensor.matmul(
                    ps_f,
                    outT[:, cc, st * P:(st + 1) * P],
                    wo_sb[:, cc, :],
                    start=(cc == 0), stop=(cc == NET - 1))
            o_sb = attn.tile([P, D], f32, tag="os", bufs=2, name="o_sb")
            nc.vector.tensor_add(o_sb, ps_f, bo_bc)
            nc.sync.dma_start(
                out=d_out.ap()[st * P:(st + 1) * P, :], in_=o_sb)

        def emit_rest(sc_ps, sh, et, jt, pvs, i):
            if jt == 0:
                pvs = psp.tile([65, 2, SCW], f32, tag="pv", bufs=1,
                               name="pv2")
            ex = attn.tile([P, 2, SHW], fp16, tag="ex", bufs=6, name="ex")
            nc.scalar.activation(ex, sc_ps, Exp)  # FD=1024
            pb = attn.tile([P, 2, SHW], fp16, tag="pb", bufs=8, name="pb")
            nc.vector.tensor_mul(pb, ex, _bcast(masks[sh][:, jt, :], 2))
            for hh in range(2):
                nc.tensor.matmul(
                    pvs[:, hh, :], v_aug[:, jt, 2 * et + hh, :],
                    pb[:, hh, :],
                    start=(jt == 0), stop=(jt == NST - 1))
            if jt == NST - 1:
                # spread the end-of-pair DVE burst over the next items so
                # the ex/pb rings (and thus the exp stream) never back up
                for off, th in zip([1, 2, 3, 4, 6],
                                   end_pair_thunks(sh, et, pvs)):
                    add_hook(i + off, th)
            return pvs

        # ---------------- issue schedule ----------------
        # Software-pipelined flat loop over all (sh, pair, jt): scores(i)
        # are issued BEFORE exp/mask/PV of item i-1 so the next pair's
        # scores never serialize behind the previous pair's drain.  Late
        # projections and final projections are spread one PSUM-group at
        # a time into the PE slack under the ScalarE-bound stream.
        proj_qk(0, 0, [0])      # only q(et0, sc0) gates the first scores
        proj_qk(1, 0)           # k et0, all s-chunks
        masks = {0: m0}

        # deferred work queue: thunks emitting ~1us of PE work each,
        # keyed by the flat item index before which they must be issued.
        items = [(sh, et, jt)
                 for sh in range(NSH) for et in range(NET)
                 for jt in range(NST)]
        hooks = {}

        def add_hook(idx, fn, *args):
            hooks.setdefault(idx, []).append((fn, args))

        def _pg(which, et, sc):
            dst = qT if which == 0 else kT
            bias = bq_sb if which == 0 else bk_sb
            ps_t = psp.tile([P, SCW], f32, tag="pp", bufs=2, name="ps_t")
            for cc in range(NET):
                nc.tensor.matmul(
                    ps_t,
                    w_sbs[which][:, cc, et * P:(et + 1) * P],
                    x_sbs[which][:, cc, sc * SCW:(sc + 1) * SCW],
                    start=(cc == 0), stop=(cc == NET - 1))
            nc.vector.tensor_scalar_add(
                dst[:, et, sc * SCW:(sc + 1) * SCW], ps_t,
                bias[:, et:et + 1])

        def load_mask_into(sh):
            masks[sh] = load_mask(sh)

        # v projection streamed one st-group per item (JIT ahead of the
        # PV matmuls, which lag scores by one item)
        for st in range(NST):
            add_hook(st, lambda s=st: proj_v(s, s + 1))
        # k (all chunks) + q (first s-chunk) for pair et must complete
        # before item (0,et,0); spread over the preceding pair's items.
        # q's s-chunks for later sh blocks are deferred until just before
        # that block starts.
        for et in range(1, NET):
            base = et * NST - 14
            add_hook(base, lambda e=et: _pg(0, e, 0))
            for g in range(4):
                add_hook(base + 2 * g + 1, lambda e=et, s=g: _pg(1, e, s))
        for sh in range(1, NSH):
            for et in range(NET):
                add_hook(sh * NET * NST - 30 + 4 * et,
                         lambda e=et, s=sh: _pg(0, e, s))
        # mask prefetch mid-way through the previous block
        for sh in range(1, NSH):
            add_hook((sh - 1) * NET * NST + 2 * NST, load_mask_into, sh)
        # final projection of block sh-1 spread over block sh's first pair
        # (starting several items in, so the rb bounce has completed)
        for sh in range(1, NSH):
            base = sh * NET * NST + 10
            for g, st in enumerate(range((sh - 1) * 4, sh * 4)):
                add_hook(base + 2 * g, final_proj_st, st)

        prev = None
        pvs = None
        for i, (sh, et, jt) in enumerate(items):
            for fn, args in hooks.get(i, []):
                fn(*args)
            c0 = sh * SHW
            sc_ps = psp.tile([P, 2, SHW], f32, tag="sc", bufs=2,
                             name="sc_ps")
            for hh in range(2):   # K=64 row-tiled pair, runs concurrent
                nc.tensor.matmul(
                    sc_ps[:, hh, :],
                    kT[hh * DK:(hh + 1) * DK, et, jt * P:(jt + 1) * P],
                    qT[hh * DK:(hh + 1) * DK, et, c0:c0 + SHW],
                    start=True, stop=True)
            if prev is not None:
                pvs = emit_rest(*prev, pvs, i)
            prev = (sc_ps, sh, et, jt)
        emit_rest(*prev, pvs, len(items))
        for i in range(len(items), len(items) + 12):  # drain tail hooks
            for fn, args in hooks.get(i, []):
                fn(*args)
        # tail: sc banks are free after the last exp — 4-way pipelined
        for k, st in enumerate(range(NST - 4, NST)):
            final_proj_st(st, tag=("pp" if k % 2 == 0 else "sc"))

    nc.compile()
    return nc

    nc.compile()
    return nc


def _get_nc():
    if "nc" not in _CACHE:
        _CACHE["nc"] = _build()
    return _CACHE["nc"]


def _preprocess(Q, K, V, mask, Wq, bq, Wk, bk, Wv, bv, Wo, bo):
    """Host-side sharding + layout marshalling (per-core input dicts)."""
    mT = np.ascontiguousarray(np.asarray(mask)[0, 0].T).astype(np.float16)
    # chunk columns by s-block: [NSH, S(j), SHW]
    mTc = np.ascontiguousarray(mT.reshape(S, NSH, SHW).transpose(1, 0, 2))
    wq_h = np.ascontiguousarray(np.asarray(Wq).T / 8.0).astype(np.float16)
    wk_h = np.ascontiguousarray(np.asarray(Wk).T).astype(np.float16)
    wv_h = np.ascontiguousarray(np.asarray(Wv).T).astype(np.float16)
    wo_h = np.ascontiguousarray(np.asarray(Wo).T).astype(np.float16)
    bq_h = np.asarray(bq, dtype=np.float32) / 8.0
    bk_h = np.asarray(bk, dtype=np.float32)
    bv_h = np.asarray(bv, dtype=np.float32)
    bo_h = np.asarray(bo, dtype=np.float32)
    Q, K, V = np.asarray(Q), np.asarray(K), np.asarray(V)
    in_maps = []
    for b in range(B):
        in_maps.append({
            "xq": np.ascontiguousarray(Q[b].T).astype(np.float16),
            "xk": np.ascontiguousarray(K[b].T).astype(np.float16),
            "xv": np.ascontiguousarray(V[b].T).astype(np.float16),
            "mskT": mTc,
            "wq": wq_h, "wk": wk_h, "wv": wv_h, "wo": wo_h,
            "bq": bq_h, "bk": bk_h, "bv": bv_h, "bo": bo_h,
        })
    return in_maps


def run(inputs: dict, trace: bool = False):
    nc = _get_nc()
    in_maps = _preprocess(**inputs)
    res = run_bass_kernel_spmd(nc, in_maps, core_ids=list(range(B)), trace=trace)
    outp = np.stack([res.results[b]["out"] for b in range(B)], axis=0)
    return outp.astype(np.float32), res


def kernel(**inputs) -> np.ndarray:
    outp, _ = run(inputs, trace=False)
    return outp


# revision 40
# speedup vs baseline: 1.1835x; 1.0177x over previous
"""Multi-head attention (B=8, S=2048, D=512, H=8) on 8 Trainium2 NeuronCores.

Strategy: pure data parallelism — one batch element per core, no collectives.

Per-core device pipeline (all matmuls fp16 with fp32 PSUM accumulation):
  1. Projections: qT/kT in transposed layout [e, s] (attention contracts
     dk on partitions), v in natural [s, e] layout augmented with a ones
     column per head (the PV matmul then also produces softmax denominators).
     Inputs arrive pre-transposed from host as X^T [c, s] fp16.  Projection
     issue is interleaved with attention so the ScalarE exp stream starts
     as soon as qT/kT for the first head pair exist (~20us in) instead of
     after the whole projection phase.
  2. Attention per (head-pair, s-block of 512): the two heads of an e-tile
     live on partitions 0-63 / 64-127, so their K=64 score matmuls are
     issued back-to-back as PE row-tiles (tile_position (0,0)/(64,0)) and
     run concurrently into one [128, 2, 512] PSUM tile (2 banks,
     double-buffered).  One FD=1024 ScalarE exp covers the pair
     (PSUM->SBUF fp16), one DVE multiply applies the 0/1 mask to both
     heads via a zero-stride broadcast AP, then two PV matmuls with [V|1]
     stationary accumulate outT rows + denominators in PSUM.  The
     pipeline is ScalarE-bound (33.5M exps/core at 1 elem/lane/cycle);
     PE and DVE have slack.
  3. Denominator rows staged to SBUF, DVE reciprocal, partition-broadcast
     via a DRAM bounce, in-place normalize, final projection with Wo.T
     (issued after the next s-block's first pair so the exp stream never
     waits on it), bias, DMA out.

Softmax note: reference softmax(where(mask==0, -1e30, s)) == exp(s)*mask
normalized — scores are O(1) so no max-subtraction is needed, and the 0/1
mask is exact in fp16. Scale 1/sqrt(dk)=1/8 is folded into Wq/bq on host.
"""
import numpy as np

import concourse.bacc as bacc
import concourse.bass as bass
import concourse.mybir as mybir
import concourse.tile as tile
from concourse.bass_utils import run_bass_kernel_spmd

B, S, D, H, DK = 8, 2048, 512, 8, 64
P = 128            # partition tile
NET = D // P       # 4 e-tiles (contraction chunks / head pairs)
NST = S // P       # 16 s-tiles / j-tiles
SCW = 512          # matmul moving free dim
NSC = S // SCW     # 4
SHW = 512          # attention s-block width
NSH = S // SHW     # 4

f32 = mybir.dt.float32
fp16 = mybir.dt.float16

_CACHE: dict = {}


def _bcast(ap, n):
    """Repeat a [P, w] AP n times along a new middle dim (stride 0)."""
    return bass.AP(tensor=ap.tensor, offset=ap.offset,
                   ap=[ap.ap[0], [0, n]] + ap.ap[1:])


def _build():
    nc = bacc.Bacc("TRN2", target_bir_lowering=False, debug=False)

    d_xq = nc.dram_tensor("xq", [D, S], fp16, kind="ExternalInput")
    d_xk = nc.dram_tensor("xk", [D, S], fp16, kind="ExternalInput")
    d_xv = nc.dram_tensor("xv", [D, S], fp16, kind="ExternalInput")
    # mask chunked by s-block on host: [NSH, S(j), SHW]
    d_mskT = nc.dram_tensor("mskT", [NSH, S, SHW], fp16, kind="ExternalInput")
    d_wq = nc.dram_tensor("wq", [D, D], fp16, kind="ExternalInput")  # Wq.T/8
    d_wk = nc.dram_tensor("wk", [D, D], fp16, kind="ExternalInput")  # Wk.T
    d_wv = nc.dram_tensor("wv", [D, D], fp16, kind="ExternalInput")  # Wv.T
    d_wo = nc.dram_tensor("wo", [D, D], fp16, kind="ExternalInput")  # Wo.T
    d_bq = nc.dram_tensor("bq", [D], f32, kind="ExternalInput")      # bq/8
    d_bk = nc.dram_tensor("bk", [D], f32, kind="ExternalInput")
    d_bv = nc.dram_tensor("bv", [D], f32, kind="ExternalInput")
    d_bo = nc.dram_tensor("bo", [D], f32, kind="ExternalInput")
    d_out = nc.dram_tensor("out", [S, D], f32, kind="ExternalOutput")
    d_rec = nc.dram_tensor("rec_dram", [H, S], f32)

    Exp = mybir.ActivationFunctionType.Exp

    with tile.TileContext(nc) as tc, \
         tc.tile_pool(name="persist", bufs=1) as persist, \
         tc.tile_pool(name="maskp", bufs=2) as maskp, \
         tc.tile_pool(name="projx", bufs=3) as projx, \
         tc.tile_pool(name="projw", bufs=3) as projw, \
         tc.tile_pool(name="attn", bufs=4) as attn, \
         tc.tile_pool(name="ps", bufs=1, space="PSUM") as psp:

        qT = persist.tile([P, NET, S], fp16)             # [e%128, et, s]
        kT = persist.tile([P, NET, S], fp16)
        v_aug = persist.tile([P, NST, H, DK + 1], fp16)  # [j%128, jt, h, d|1]
        outT = persist.tile([P, NET, S], fp16)           # [hd%128, et, s]
        denom = persist.tile([P, NSH, 64], f32)
        bq_sb = persist.tile([P, NET], f32)
        bk_sb = persist.tile([P, NET], f32)
        bv_bc = persist.tile([P, D], f32)
        wo_sb = persist.tile([P, NET, D], fp16)
        bo_bc = persist.tile([P, D], f32)
        warm = persist.tile([P, 2], f32)

        # small constants / biases first, then the exp table preload runs
        # while the big input DMAs stream.
        nc.sync.dma_start(out=bq_sb, in_=d_bq.ap().rearrange("(cc p) -> p cc", p=P))
        nc.sync.dma_start(out=bk_sb, in_=d_bk.ap().rearrange("(cc p) -> p cc", p=P))
        nc.sync.dma_start(
            out=bv_bc,
            in_=bass.AP(tensor=d_bv.ap().tensor, offset=0, ap=[[0, P], [1, D]]))
        nc.vector.memset(v_aug[:, :, :, DK:DK + 1], 1.0)
        nc.scalar.activation(warm[:, 0:1], bq_sb[:, 0:1], Exp)  # table preload

        # ---------------- input DMAs (issue order = criticality) --------
        # q and k first (gate the first scores), then v streamed JIT in
        # (cc, st-quarter) chunks so early PV matmuls never wait on the
        # whole xv transfer, then the first mask block.
        x_sbs, w_sbs, x_aps = [], [], []
        for which, (d_x, d_w) in enumerate(
                [(d_xq, d_wq), (d_xk, d_wk), (d_xv, d_wv)]):
            w_sb = projw.tile([P, NET, D], fp16, tag="w", name=f"w{which}")
            x_sb = projx.tile([P, NET, S], fp16, tag="x", name=f"x{which}")
            x_sbs.append(x_sb)
            w_sbs.append(w_sb)
            x_aps.append(d_x.ap().rearrange("(cc p) s -> p cc s", p=P))

        def dma_w(which):
            nc.sync.dma_start(
                out=w_sbs[which],
                in_=[d_wq, d_wk, d_wv][which].ap().rearrange(
                    "(cc p) e -> p cc e", p=P))

        def dma_x(which, lo, hi):
            for cc in range(NET):
                nc.sync.dma_start(out=x_sbs[which][:, cc, lo:hi],
                                  in_=x_aps[which][:, cc, lo:hi])

        m0 = maskp.tile([P, NST, SHW], fp16, tag="m", name="m0")
        # only q(et0, sc0) gates the first scores: 1MB of xq first, then
        # all of xk; xv quarters interleaved with mask rows so PV matmuls
        # and mask multiplies both stream JIT; deferred xq after.
        dma_w(0)
        dma_x(0, 0, SCW)
        dma_w(1)
        dma_x(1, 0, S)
        dma_w(2)
        msk0_ap = d_mskT.ap()[0].rearrange("(jt p) w -> p jt w", p=P)
        for q4 in range(4):
            dma_x(2, q4 * SCW, (q4 + 1) * SCW)
            nc.sync.dma_start(out=m0[:, q4 * 4:(q4 + 1) * 4, :],
                              in_=msk0_ap[:, q4 * 4:(q4 + 1) * 4, :])
        dma_x(0, SCW, S)
        nc.sync.dma_start(
            out=wo_sb, in_=d_wo.ap().rearrange("(cc p) e -> p cc e", p=P))
        nc.sync.dma_start(
            out=bo_bc,
            in_=bass.AP(tensor=d_bo.ap().tensor, offset=0, ap=[[0, P], [1, D]]))

        def load_mask(sh):
            m = maskp.tile([P, NST, SHW], fp16, tag="m", name=f"m{sh}")
            nc.sync.dma_start(
                out=m, in_=d_mskT.ap()[sh].rearrange("(jt p) w -> p jt w", p=P))
            return m

        def proj_qk(which, et, scs=None):
            dst = qT if which == 0 else kT
            bias = bq_sb if which == 0 else bk_sb
            for sc in (range(NSC) if scs is None else scs):
                ps_t = psp.tile([P, SCW], f32, tag="pp", bufs=2, name="ps_t")
                for cc in range(NET):
                    nc.tensor.matmul(
                        ps_t,
                        w_sbs[which][:, cc, et * P:(et + 1) * P],
                        x_sbs[which][:, cc, sc * SCW:(sc + 1) * SCW],
                        start=(cc == 0), stop=(cc == NET - 1))
                nc.vector.tensor_scalar_add(
                    dst[:, et, sc * SCW:(sc + 1) * SCW], ps_t,
                    bias[:, et:et + 1])

        def proj_v(st_lo, st_hi):
            for st in range(st_lo, st_hi):
                ps_t = psp.tile([P, SCW], f32, tag="pp", bufs=2, name="ps_t")
                for cc in range(NET):
                    nc.tensor.matmul(
                        ps_t,
                        x_sbs[2][:, cc, st * P:(st + 1) * P],
                        w_sbs[2][:, cc, :],
                        start=(cc == 0), stop=(cc == NET - 1))
                nc.vector.tensor_add(
                    v_aug[:, st, :, 0:DK],
                    ps_t.rearrange("p (h d) -> p h d", h=H),
                    bv_bc.rearrange("p (h d) -> p h d", h=H))

        def end_pair_thunks(sh, et, pvs):
            c0 = sh * SHW

            def cast(hh):
                ro = hh * DK
                nc.vector.tensor_copy(
                    outT[ro:ro + DK, et, c0:c0 + SHW], pvs[0:DK, hh, :])

            def dstage():
                dst_t = attn.tile([65, 2, SCW], f32, tag="dst", bufs=1,
                                  name="dst_t")
                nc.vector.tensor_copy(dst_t[64:65, :, :], pvs[64:65, :, :])
                nc.gpsimd.dma_start(
                    out=denom[et * 32:et * 32 + 16, sh, :],
                    in_=dst_t[64:65, :, :])

            def recb():
                rec = attn.tile([16, 64], f32, tag="rec", bufs=2, name="rec")
                nc.vector.reciprocal(rec, denom[et * 32:et * 32 + 16, sh, :])
                nc.sync.dma_start(
                    out=d_rec.ap()[2 * et:2 * et + 2, c0:c0 + SHW], in_=rec)

            def norm():
                rb = attn.tile([P, SHW], f32, tag="rb", bufs=2, name="rb")
                nc.gpsimd.dma_start(
                    out=rb[0:64, :],
                    in_=bass.AP(tensor=d_rec.ap().tensor,
                                offset=(2 * et) * S + c0,
                                ap=[[0, 64], [1, SHW]]))
                nc.gpsimd.dma_start(
                    out=rb[64:128, :],
                    in_=bass.AP(tensor=d_rec.ap().tensor,
                                offset=(2 * et + 1) * S + c0,
                                ap=[[0, 64], [1, SHW]]))
                nc.vector.tensor_mul(outT[:, et, c0:c0 + SHW],
                                     outT[:, et, c0:c0 + SHW], rb)

            return [lambda: cast(0), lambda: cast(1), dstage, recb, norm]

        def final_proj_st(st):
            ps_f = psp.tile([P, D], f32, tag="pp", bufs=2, name="ps_f")
            for cc in range(NET):
                nc.tensor.matmul(
                    ps_f,
                    outT[:, cc, st * P:(st + 1) * P],
                    wo_sb[:, cc, :],
                    start=(cc == 0), stop=(cc == NET - 1))
            o_sb = attn.tile([P, D], f32, tag="os", bufs=2, name="o_sb")
            nc.vector.tensor_add(o_sb, ps_f, bo_bc)
            nc.sync.dma_start(
                out=d_out.ap()[st * P:(st + 1) * P, :], in_=o_sb)

        def emit_rest(sc_ps, sh, et, jt, pvs, i):
            if jt == 0:
                pvs = psp.tile([65, 2, SCW], f32, tag="pv", bufs=1,
                               name="pv2")
            ex = attn.tile([P, 2, SHW], fp16, tag="ex", bufs=6, name="ex")
            nc.scalar.activation(ex, sc_ps, Exp)  # FD=1024
            pb = attn.tile([P, 2, SHW], fp16, tag="pb", bufs=8, name="pb")
            nc.vector.tensor_mul(pb, ex, _bcast(masks[sh][:, jt, :], 2))
            for hh in range(2):
                nc.tensor.matmul(
                    pvs[:, hh, :], v_aug[:, jt, 2 * et + hh, :],
                    pb[:, hh, :],
                    start=(jt == 0), stop=(jt == NST - 1))
            if jt == NST - 1:
                # spread the end-of-pair DVE burst over the next items so
                # the ex/pb rings (and thus the exp stream) never back up
                for off, th in zip([1, 2, 3, 5, 7],
                                   end_pair_thunks(sh, et, pvs)):
                    add_hook(i + off, th)
            return pvs

        # ---------------- issue schedule ----------------
        # Software-pipelined flat loop over all (sh, pair, jt): scores(i)
        # are issued BEFORE exp/mask/PV of item i-1 so the next pair's
        # scores never serialize behind the previous pair's drain.  Late
        # projections and final projections are spread one PSUM-group at
        # a time into the PE slack under the ScalarE-bound stream.
        proj_qk(0, 0, [0])      # only q(et0, sc0) gates the first scores
        proj_qk(1, 0)           # k et0, all s-chunks
        masks = {0: m0}

        # deferred work queue: thunks emitting ~1us of PE work each,
        # keyed by the flat item index before which they must be issued.
        items = [(sh, et, jt)
                 for sh in range(NSH) for et in range(NET)
                 for jt in range(NST)]
        hooks = {}

        def add_hook(idx, fn, *args):
            hooks.setdefault(idx, []).append((fn, args))

        def _pg(which, et, sc):
            dst = qT if which == 0 else kT
            bias = bq_sb if which == 0 else bk_sb
            ps_t = psp.tile([P, SCW], f32, tag="pp", bufs=2, name="ps_t")
            for cc in range(NET):
                nc.tensor.matmul(
                    ps_t,
                    w_sbs[which][:, cc, et * P:(et + 1) * P],
                    x_sbs[which][:, cc, sc * SCW:(sc + 1) * SCW],
                    start=(cc == 0), stop=(cc == NET - 1))
            nc.vector.tensor_scalar_add(
                dst[:, et, sc * SCW:(sc + 1) * SCW], ps_t,
                bias[:, et:et + 1])

        def load_mask_into(sh):
            masks[sh] = load_mask(sh)

        # v projection streamed one st-group per item (JIT ahead of the
        # PV matmuls, which lag scores by one item)
        for st in range(NST):
            add_hook(st, lambda s=st: proj_v(s, s + 1))
        # k (all chunks) + q (first s-chunk) for pair et must complete
        # before item (0,et,0); spread over the preceding pair's items.
        # q's s-chunks for later sh blocks are deferred until just before
        # that block starts.
        for et in range(1, NET):
            base = et * NST - 14
            add_hook(base, lambda e=et: _pg(0, e, 0))
            for g in range(4):
                add_hook(base + 2 * g + 1, lambda e=et, s=g: _pg(1, e, s))
        for sh in range(1, NSH):
            for et in range(NET):
                add_hook(sh * NET * NST - 14 + 3 * et,
                         lambda e=et, s=sh: _pg(0, e, s))
        # mask prefetch mid-way through the previous block
        for sh in range(1, NSH):
            add_hook((sh - 1) * NET * NST + 2 * NST, load_mask_into, sh)
        # final projection of block sh-1 spread over block sh's first pair
        # (starting several items in, so the rb bounce has completed)
        for sh in range(1, NSH):
            base = sh * NET * NST + 10
            for g, st in enumerate(range((sh - 1) * 4, sh * 4)):
                add_hook(base + 2 * g, final_proj_st, st)

        prev = None
        pvs = None
        for i, (sh, et, jt) in enumerate(items):
            for fn, args in hooks.get(i, []):
                fn(*args)
            c0 = sh * SHW
            sc_ps = psp.tile([P, 2, SHW], f32, tag="sc", bufs=2,
                             name="sc_ps")
            for hh in range(2):   # K=64 row-tiled pair, runs concurrent
                nc.tensor.matmul(
                    sc_ps[:, hh, :],
                    kT[hh * DK:(hh + 1) * DK, et, jt * P:(jt + 1) * P],
                    qT[hh * DK:(hh + 1) * DK, et, c0:c0 + SHW],
                    start=True, stop=True)
            if prev is not None:
                pvs = emit_rest(*prev, pvs, i)
            prev = (sc_ps, sh, et, jt)
        emit_rest(*prev, pvs, len(items))
        for i in range(len(items), len(items) + 12):  # drain tail hooks
            for fn, args in hooks.get(i, []):
                fn(*args)
        for st in range(NST - 4, NST):
            final_proj_st(st)

    nc.compile()
    return nc

    nc.compile()
    return nc


def _get_nc():
    if "nc" not in _CACHE:
        _CACHE["nc"] = _build()
    return _CACHE["nc"]


def _preprocess(Q, K, V, mask, Wq, bq, Wk, bk, Wv, bv, Wo, bo):
    """Host-side sharding + layout marshalling (per-core input dicts)."""
    mT = np.ascontiguousarray(np.asarray(mask)[0, 0].T).astype(np.float16)
    # chunk columns by s-block: [NSH, S(j), SHW]
    mTc = np.ascontiguousarray(mT.reshape(S, NSH, SHW).transpose(1, 0, 2))
    wq_h = np.ascontiguousarray(np.asarray(Wq).T / 8.0).astype(np.float16)
    wk_h = np.ascontiguousarray(np.asarray(Wk).T).astype(np.float16)
    wv_h = np.ascontiguousarray(np.asarray(Wv).T).astype(np.float16)
    wo_h = np.ascontiguousarray(np.asarray(Wo).T).astype(np.float16)
    bq_h = np.asarray(bq, dtype=np.float32) / 8.0
    bk_h = np.asarray(bk, dtype=np.float32)
    bv_h = np.asarray(bv, dtype=np.float32)
    bo_h = np.asarray(bo, dtype=np.float32)
    Q, K, V = np.asarray(Q), np.asarray(K), np.asarray(V)
    in_maps = []
    for b in range(B):
        in_maps.append({
            "xq": np.ascontiguousarray(Q[b].T).astype(np.float16),
            "xk": np.ascontiguousarray(K[b].T).astype(np.float16),
            "xv": np.ascontiguousarray(V[b].T).astype(np.float16),
            "mskT": mTc,
            "wq": wq_h, "wk": wk_h, "wv": wv_h, "wo": wo_h,
            "bq": bq_h, "bk": bk_h, "bv": bv_h, "bo": bo_h,
        })
    return in_maps


def run(inputs: dict, trace: bool = False):
    nc = _get_nc()
    in_maps = _preprocess(**inputs)
    res = run_bass_kernel_spmd(nc, in_maps, core_ids=list(range(B)), trace=trace)
    outp = np.stack([res.results[b]["out"] for b in range(B)], axis=0)
    return outp.astype(np.float32), res


def kernel(**inputs) -> np.ndarray:
    outp, _ = run(inputs, trace=False)
    return outp
